# revision 8
# baseline (speedup 1.0000x reference)
"""Trainium2 Bass kernel for nn_Blur2: depthwise 4x4 blur (upfirdn2d-style,
pad=(2,1), unit stride) over input [8, 128, 256, 256] f32.

Strategy: pure data parallel over the 1024 independent (n, c) planes --
128 planes per NeuronCore. Within a plane the 2D 16-tap conv runs on the
tensor engine as banded matmuls: the H-direction conv is the contraction
(banded Toeplitz fp16 weights, image rows on partitions) and the
W-direction conv is 4 shifted slices of the moving operand accumulated
into the same PSUM tile (clipped column ranges encode the zero padding,
clipped weight bands encode the H padding).

Precision: tolerance is rel 2e-2 at output scale ~6.5, so the input is
quantized to a SINGLE fp16 (worst-case output err ~1.1e-2 abs, measured
~2e-3 rel) -- this halves both the matmul count and the input DMA
traffic vs an exact hi/lo split. The output is likewise stored as fp16
(adds <= 2^-11 rel) and upcast to f32 on host.

DMA: planes are packed in OCTS on host -- each DRAM row holds 8 planes'
fp16 data = 4 KB -- so every DMA moves >=4KB per partition, which is
the difference between ~100 GB/s and ~340 GB/s per core on TRN2. The
output uses a 260-row-per-oct DRAM layout (junk rows at 127, 253..255)
so both store DMAs are exactly 128 partitions: the HWDGE splits an
SBUF->DRAM DMA across its 16 SDMA engines only when the partition count
divides into 16 chunks. Loads go on the sync HWDGE ring, stores on the
scalar HWDGE ring.
"""
import sys

for _p in ("/opt/trn_rl_repo", "/opt/pypackages"):
    if _p not in sys.path:
        sys.path.insert(0, _p)

import contextlib

import numpy as np


def _install_ntff_hook_shim():
    """The agent image's antenv lacks axon_hooks, which bass_utils needs
    for trace=True under axon. Provide it in sys.modules, backed by
    trn_agent_boot's ctypes NTFF shim."""
    import types

    if "antenv.axon_hooks" in sys.modules:
        return
    mod = types.ModuleType("antenv.axon_hooks")
    state = {"hook": None, "tried": False}

    def set_axon_ntff_profile_hook(hook):
        state["hook"] = hook

    def get_axon_ntff_profile_hook():
        if state["hook"] is None and not state["tried"]:
            state["tried"] = True
            try:
                from trn_agent_boot.trn_boot import _ntff_profile_via_ctypes

                state["hook"] = _ntff_profile_via_ctypes("/opt/axon/libaxon_pjrt.so")
            except Exception:
                state["hook"] = None
        return state["hook"]

    mod.set_axon_ntff_profile_hook = set_axon_ntff_profile_hook
    mod.get_axon_ntff_profile_hook = get_axon_ntff_profile_hook
    sys.modules["antenv.axon_hooks"] = mod
    try:
        import antenv

        antenv.axon_hooks = mod
    except ImportError:
        pass


_install_ntff_hook_shim()

import concourse.bacc as bacc
import concourse.tile as tile
from concourse import mybir
from concourse.bass_utils import run_bass_kernel_spmd

N_CORES = 8
H = W = 256
PLANES = 1024 // N_CORES  # 128 per core
Q = 8  # planes packed per SBUF/DRAM row (4 KB of fp16)
NQUAD = PLANES // Q  # 16 oct-groups per core
SEC = W  # one plane's section in a packed row: 256 fp16

# M-tile layout along H per plane:
#   tile A: out rows [0, 127)   from x rows [0, 128)
#   tile B: out rows [127, 252) from x rows [125, 253)
#   remainder: out rows [252, 256) from x rows [250, 256), stacked across
#   groups of RG=16 octs (96 partitions, 64 out rows per plane-slot)
MA, MB = 127, 125
RG = 16

# per W-shift i: out cols [wl, wh), reading x cols [cl, ch)  (tap = w-2+i)
SHIFT_RANGES = {
    0: (2, 256, 0, 254),
    1: (1, 256, 0, 255),
    2: (0, 256, 0, 256),
    3: (0, 255, 1, 256),
}
SHIFT_ORDER = [2, 0, 1, 3]  # full-range shift first so start=True covers all


def _make_weights(wk: np.ndarray):
    """wk: flipped 4x4 kernel. Packed fp16 weights, one 128-col matrix per
    W-shift (cols padded with zeros past MA/MB so NumWeights==128 enables
    the PE Fast-Weight-Load path): wa/wb [128, 4*128], wr [96, 4*64]
    (block-diag 16x(6->4))."""
    wa = np.zeros((128, 4, 128), np.float32)
    for k in range(128):
        for m in range(MA):
            d = k - m + 2
            if 0 <= d <= 3:
                wa[k, :, m] = wk[d, :]
    wb = np.zeros((128, 4, 128), np.float32)
    for k in range(128):
        for m in range(MB):
            d = k - m
            if 0 <= d <= 3:
                wb[k, :, m] = wk[d, :]
    wr = np.zeros((RG * 6, 4, RG * 4), np.float32)
    for b in range(RG):
        for r in range(6):
            for c in range(4):
                d = r - c
                if 0 <= d <= 3:
                    wr[6 * b + r, :, 4 * b + c] = wk[d, :]
    return (
        wa.reshape(128, 4 * 128).astype(np.float16),
        wb.reshape(128, 4 * 128).astype(np.float16),
        wr.reshape(RG * 6, 4 * RG * 4).astype(np.float16),
    )


def _build_program(nquad: int = NQUAD):
    nc = bacc.Bacc("TRN2", target_bir_lowering=False, debug=False)
    f16, f32 = mybir.dt.float16, mybir.dt.float32

    d_xs = nc.dram_tensor("xs", [nquad, H, Q * SEC], f16, kind="ExternalInput").ap()
    d_wa = nc.dram_tensor("wa", [128, 4 * 128], f16, kind="ExternalInput").ap()
    d_wb = nc.dram_tensor("wb", [128, 4 * 128], f16, kind="ExternalInput").ap()
    d_wr = nc.dram_tensor("wr", [RG * 6, 4 * RG * 4], f16, kind="ExternalInput").ap()
    d_out = nc.dram_tensor("out", [nquad, H + 4, Q * W], f16, kind="ExternalOutput").ap()

    rem_groups = [(s, min(RG, nquad - s)) for s in range(0, nquad, RG)]

    with tile.TileContext(nc) as tc, contextlib.ExitStack() as ctx:
        wpool = ctx.enter_context(tc.tile_pool(name="wpool", bufs=1))
        xin = ctx.enter_context(tc.tile_pool(name="xin", bufs=6))
        xinr = ctx.enter_context(tc.tile_pool(name="xinr", bufs=2))
        psum = ctx.enter_context(tc.tile_pool(name="psum", bufs=2, space="PSUM"))
        outp = ctx.enter_context(tc.tile_pool(name="outp", bufs=6))
        outr = ctx.enter_context(tc.tile_pool(name="outr", bufs=2))

        # PE warmup: ~20 junk matmuls with no data dependencies, issued
        # before any real work. They run during the DMA ramp (t~5-10us)
        # and lift the HAM clock gate to 2.4 GHz before the real stream
        # starts. Results land in a scratch psum slot and are discarded;
        # any garbage/NaN is overwritten later because every bank's first
        # real matmul runs with start=True.
        warm = wpool.tile([128, W], f16, tag="warm")
        nc.gpsimd.memset(warm[:], 0.0)
        psW = psum.tile([128, W], f32, tag="psA")
        for _ in range(20):
            nc.tensor.matmul(
                psW[:, :], warm[:, :128], warm[:, :],
                start=True, stop=True, skip_group_check=True,
            )

        t_wa = wpool.tile([128, 4 * 128], f16, tag="wa")
        nc.scalar.dma_start(out=t_wa[:], in_=d_wa)
        t_wb = wpool.tile([128, 4 * 128], f16, tag="wb")
        nc.scalar.dma_start(out=t_wb[:], in_=d_wb)
        t_wr = wpool.tile([RG * 6, 4 * RG * 4], f16, tag="wr")
        nc.scalar.dma_start(out=t_wr[:], in_=d_wr)

        def conv_mms(ps, wt, xt, xrows, hoff):
            """4 shifts x 4 oct-planes accumulating matmuls into the
            half-oct psum tile ps [128, 4*W]. Quad-major order (all 4
            shifts of a plane back to back) so the first matmuls only
            depend on the first plane's columns being loaded. Each quad's
            column range is initialized by its full-range shift
            (start=True) and finalized by its last shift (stop=True).
            hoff selects planes [hoff, hoff+4)."""
            for q in range(4):
                for i in SHIFT_ORDER:
                    wl, wh, cl, ch = SHIFT_RANGES[i]
                    nc.tensor.matmul(
                        ps[:128, q * W + wl : q * W + wh],
                        wt[:xrows, i * 128 : i * 128 + 128],
                        xt[:xrows, (hoff + q) * SEC + cl : (hoff + q) * SEC + ch],
                        start=(i == SHIFT_ORDER[0]),
                        stop=(i == SHIFT_ORDER[-1]),
                        skip_group_check=True,
                    )

        def emit_half(o, ps, hoff, alt):
            """cast-copy psum half-oct tile [128, 4*W] f32 into fp16 out
            tile o at cols [hoff*W, (hoff+4)*W), split across engines."""
            hw = 2 * W
            base = hoff * W
            if alt:
                nc.scalar.copy(o[:, base : base + hw], ps[:, :hw])
                nc.vector.tensor_copy(o[:, base + hw : base + 4 * W], ps[:, hw:])
            else:
                nc.vector.tensor_copy(o[:, base : base + hw], ps[:, :hw])
                nc.scalar.copy(o[:, base + hw : base + 4 * W], ps[:, hw:])

        ri = 0
        for g in range(nquad):
            ta = xin.tile([128, Q * SEC], f16, tag="ta")
            if g == 0:
                # split the very first load by planes so the first matmul
                # group (plane 0) starts after only 64 KB has landed
                for c0, c1 in ((0, 1), (1, 2), (2, 4), (4, 8)):
                    nc.sync.dma_start(
                        out=ta[:, c0 * SEC : c1 * SEC],
                        in_=d_xs[g, 0:128, c0 * SEC : c1 * SEC],
                    )
            else:
                nc.sync.dma_start(out=ta[:], in_=d_xs[g, 0:128, :])
            tb = xin.tile([128, Q * SEC], f16, tag="tb")
            nc.sync.dma_start(out=tb[:], in_=d_xs[g, 125:253, :])

            # the last oct's stores ride the sync ring (idle once loads done)
            ring = nc.sync if g == nquad - 1 else nc.scalar
            oa = outp.tile([128, Q * W], f16, tag="oa")
            for hoff in (0, 4):
                psA = psum.tile([128, 4 * W], f32, tag="psA")
                conv_mms(psA, t_wa, ta, 128, hoff)
                emit_half(oa, psA, hoff, alt=(g + hoff) % 2 == 0)
            ring.dma_start(out=d_out[g, 0:128, :], in_=oa[:])

            ob = outp.tile([128, Q * W], f16, tag="ob")
            for hoff in (0, 4):
                psB = psum.tile([128, 4 * W], f32, tag="psB")
                conv_mms(psB, t_wb, tb, 128, hoff)
                emit_half(ob, psB, hoff, alt=(g + hoff) % 2 == 1)
                if g == nquad - 1:
                    ring.dma_start(
                        out=d_out[g, 128:256, hoff * W : (hoff + 4) * W],
                        in_=ob[:, hoff * W : (hoff + 4) * W],
                    )
            if g != nquad - 1:
                ring.dma_start(out=d_out[g, 128:256, :], in_=ob[:])

            # stacked remainder: input rows come straight from DRAM, so
            # emit early (octs 2, 4, ...) to keep them off the kernel tail
            if ri < len(rem_groups) and g == min(2 * (ri + 1), nquad - 1):
                s, gsz = rem_groups[ri]
                ri += 1
                tr = xinr.tile([RG * 6, Q * SEC], f16, tag="tr")
                nc.sync.dma_start(
                    out=tr[: 6 * gsz, :], in_=d_xs[s : s + gsz, 250:256, :]
                )
                orr = outr.tile([RG * 4, Q * W], f16, tag="orr")
                for hoff in (0, 4):
                    psR = psum.tile([RG * 4, 4 * W], f32, tag="psA")
                    last = (SHIFT_ORDER[-1], 3)
                    for i in SHIFT_ORDER:
                        wl, wh, cl, ch = SHIFT_RANGES[i]
                        lhsT = t_wr[: 6 * gsz, i * RG * 4 : i * RG * 4 + 4 * gsz]
                        for q in range(4):
                            nc.tensor.matmul(
                                psR[: 4 * gsz, q * W + wl : q * W + wh],
                                lhsT,
                                tr[: 6 * gsz, (hoff + q) * SEC + cl : (hoff + q) * SEC + ch],
                                start=(i == SHIFT_ORDER[0] and q % 2 == 0),
                                stop=((i, q) == last),
                                skip_group_check=True,
                            )
                    if (g + hoff) % 2 == 0:
                        nc.scalar.copy(
                            orr[: 4 * gsz, hoff * W : (hoff + 4) * W], psR[: 4 * gsz, :]
                        )
                    else:
                        nc.vector.tensor_copy(
                            orr[: 4 * gsz, hoff * W : (hoff + 4) * W], psR[: 4 * gsz, :]
                        )
                nc.scalar.dma_start(
                    out=d_out[s : s + gsz, H : H + 4, :], in_=orr[: 4 * gsz, :]
                )

    nc.compile()
    return nc


_CACHE = {}


def _get_program(nquad: int = NQUAD):
    if nquad not in _CACHE:
        _CACHE[nquad] = _build_program(nquad)
    return _CACHE[nquad]


def _run(x: np.ndarray, wk: np.ndarray, trace: bool = False):
    """x: [P, 256, 256] f32 full stack of planes (P divisible by 8*Q),
    wk: flipped 4x4 kernel. Returns ([P, 256, 256] f32, exec_time_ns|None)."""
    P = x.shape[0]
    qper = P // (N_CORES * Q)
    xs = x.astype(np.float16)  # [P, 256, 256]
    # oct-pack: [P/Q, Q, H, SEC] -> [P/Q, H, Q, SEC] -> [P/Q, H, Q*SEC]
    xsq = (
        xs.reshape(P // Q, Q, H, SEC)
        .transpose(0, 2, 1, 3)
        .reshape(P // Q, H, Q * SEC)
    )

    wa, wb, wr = _make_weights(wk)
    nc = _get_program(qper)

    in_maps = [
        {
            "xs": np.ascontiguousarray(xsq[c * qper : (c + 1) * qper]),
            "wa": wa,
            "wb": wb,
            "wr": wr,
        }
        for c in range(N_CORES)
    ]
    res = run_bass_kernel_spmd(nc, in_maps, list(range(N_CORES)), trace=trace)
    outq = np.concatenate([r["out"] for r in res.results], axis=0)  # [P/Q, H+4, Q*W]
    outq = np.concatenate(
        [outq[:, 0:127], outq[:, 128:253], outq[:, 256:260]], axis=1
    )  # drop junk rows -> [P/Q, 256, Q*W]
    out = (
        outq.reshape(P // Q, H, Q, W)
        .transpose(0, 2, 1, 3)
        .reshape(P, H, W)
        .astype(np.float32)
    )
    return np.ascontiguousarray(out), res.exec_time_ns


def kernel(input: np.ndarray, kernel: np.ndarray) -> np.ndarray:
    x = np.asarray(input, dtype=np.float32)
    k = np.asarray(kernel, dtype=np.float32)
    n, c, h, w = x.shape
    wk = np.flip(k, (0, 1)).copy()  # correlation weights
    out, _ = _run(x.reshape(n * c, h, w), wk, trace=False)
    return out.reshape(n, c, h, w)


# revision 11
# speedup vs baseline: 1.0220x; 1.0220x over previous
"""Trainium2 Bass kernel for nn_Blur2: depthwise 4x4 blur (upfirdn2d-style,
pad=(2,1), unit stride) over input [8, 128, 256, 256] f32.

Strategy: pure data parallel over the 1024 independent (n, c) planes --
128 planes per NeuronCore. Within a plane the 2D 16-tap conv runs on the
tensor engine as banded matmuls: the H-direction conv is the contraction
(banded Toeplitz fp16 weights, image rows on partitions) and the
W-direction conv is 4 shifted slices of the moving operand accumulated
into the same PSUM tile (clipped column ranges encode the zero padding,
clipped weight bands encode the H padding).

Precision: tolerance is rel 2e-2 at output scale ~6.5, so the input is
quantized to a SINGLE fp16 (worst-case output err ~1.1e-2 abs, measured
~2e-3 rel) -- this halves both the matmul count and the input DMA
traffic vs an exact hi/lo split. The output is likewise stored as fp16
(adds <= 2^-11 rel) and upcast to f32 on host.

DMA: planes are packed in OCTS on host -- each DRAM row holds 8 planes'
fp16 data = 4 KB -- so every DMA moves >=4KB per partition, which is
the difference between ~100 GB/s and ~340 GB/s per core on TRN2. The
output uses a 260-row-per-oct DRAM layout (junk rows at 127, 253..255)
so both store DMAs are exactly 128 partitions: the HWDGE splits an
SBUF->DRAM DMA across its 16 SDMA engines only when the partition count
divides into 16 chunks. Loads go on the sync HWDGE ring, stores on the
scalar HWDGE ring; the last oct's stores ride the by-then-idle sync
ring to shorten the kernel tail.

Measured on 8 cores: HW exec ~134 us (PE ~87% busy, gapless at its
109 ns/matmul streaming floor for 1056 N~255 matmuls; ~11.5 us fixed
startup preamble + ~5 us tail), max rel err 5.2e-4 vs the f32 oracle
(tolerance 2e-2). Post-mortems of rejected faster-looking schemes: a
separable two-stage formulation (H-conv then W-conv on y^T, 2x fewer
streamed columns) measured 190-193 us because every matmul swaps its
stationary operand and each LDWEIGHTS is ~98 ns SERIAL with this
toolchain (walrus runs with --enable-ldw-opt=false, so the background
weight buffer is never used) plus HAM clock oscillation from PE
micro-idles; fp8-single fails the tolerance; DVE transposes cannot
cross 32-partition banks. Shift-major matmul order (one stationary
shared by 4 consecutive matmuls) is ~2 us faster than quad-major.
"""
import sys

for _p in ("/opt/trn_rl_repo", "/opt/pypackages"):
    if _p not in sys.path:
        sys.path.insert(0, _p)

import contextlib

import numpy as np


def _install_ntff_hook_shim():
    """The agent image's antenv lacks axon_hooks, which bass_utils needs
    for trace=True under axon. Provide it in sys.modules, backed by
    trn_agent_boot's ctypes NTFF shim."""
    import types

    if "antenv.axon_hooks" in sys.modules:
        return
    mod = types.ModuleType("antenv.axon_hooks")
    state = {"hook": None, "tried": False}

    def set_axon_ntff_profile_hook(hook):
        state["hook"] = hook

    def get_axon_ntff_profile_hook():
        if state["hook"] is None and not state["tried"]:
            state["tried"] = True
            try:
                from trn_agent_boot.trn_boot import _ntff_profile_via_ctypes

                state["hook"] = _ntff_profile_via_ctypes("/opt/axon/libaxon_pjrt.so")
            except Exception:
                state["hook"] = None
        return state["hook"]

    mod.set_axon_ntff_profile_hook = set_axon_ntff_profile_hook
    mod.get_axon_ntff_profile_hook = get_axon_ntff_profile_hook
    sys.modules["antenv.axon_hooks"] = mod
    try:
        import antenv

        antenv.axon_hooks = mod
    except ImportError:
        pass


_install_ntff_hook_shim()

import concourse.bacc as bacc
import concourse.tile as tile
from concourse import mybir
from concourse.bass_utils import run_bass_kernel_spmd

N_CORES = 8
H = W = 256
PLANES = 1024 // N_CORES  # 128 per core
Q = 8  # planes packed per SBUF/DRAM row (4 KB of fp16)
NQUAD = PLANES // Q  # 16 oct-groups per core
SEC = W  # one plane's section in a packed row: 256 fp16

# M-tile layout along H per plane:
#   tile A: out rows [0, 127)   from x rows [0, 128)
#   tile B: out rows [127, 252) from x rows [125, 253)
#   remainder: out rows [252, 256) from x rows [250, 256), stacked across
#   groups of RG=16 octs (96 partitions, 64 out rows per plane-slot)
MA, MB = 127, 125
RG = 16

# per W-shift i: out cols [wl, wh), reading x cols [cl, ch)  (tap = w-2+i)
SHIFT_RANGES = {
    0: (2, 256, 0, 254),
    1: (1, 256, 0, 255),
    2: (0, 256, 0, 256),
    3: (0, 255, 1, 256),
}
SHIFT_ORDER = [2, 0, 1, 3]  # full-range shift first so start=True covers all


def _make_weights(wk: np.ndarray):
    """wk: flipped 4x4 kernel. Packed fp16 weights, one 128-col matrix per
    W-shift (cols padded with zeros past MA/MB so NumWeights==128 enables
    the PE Fast-Weight-Load path): wa/wb [128, 4*128], wr [96, 4*64]
    (block-diag 16x(6->4))."""
    wa = np.zeros((128, 4, 128), np.float32)
    for k in range(128):
        for m in range(MA):
            d = k - m + 2
            if 0 <= d <= 3:
                wa[k, :, m] = wk[d, :]
    wb = np.zeros((128, 4, 128), np.float32)
    for k in range(128):
        for m in range(MB):
            d = k - m
            if 0 <= d <= 3:
                wb[k, :, m] = wk[d, :]
    wr = np.zeros((RG * 6, 4, RG * 4), np.float32)
    for b in range(RG):
        for r in range(6):
            for c in range(4):
                d = r - c
                if 0 <= d <= 3:
                    wr[6 * b + r, :, 4 * b + c] = wk[d, :]
    return (
        wa.reshape(128, 4 * 128).astype(np.float16),
        wb.reshape(128, 4 * 128).astype(np.float16),
        wr.reshape(RG * 6, 4 * RG * 4).astype(np.float16),
    )


def _build_program(nquad: int = NQUAD):
    nc = bacc.Bacc("TRN2", target_bir_lowering=False, debug=False)
    f16, f32 = mybir.dt.float16, mybir.dt.float32

    d_xs = nc.dram_tensor("xs", [nquad, H, Q * SEC], f16, kind="ExternalInput").ap()
    d_wa = nc.dram_tensor("wa", [128, 4 * 128], f16, kind="ExternalInput").ap()
    d_wb = nc.dram_tensor("wb", [128, 4 * 128], f16, kind="ExternalInput").ap()
    d_wr = nc.dram_tensor("wr", [RG * 6, 4 * RG * 4], f16, kind="ExternalInput").ap()
    d_out = nc.dram_tensor("out", [nquad, H + 4, Q * W], f16, kind="ExternalOutput").ap()

    rem_groups = [(s, min(RG, nquad - s)) for s in range(0, nquad, RG)]

    with tile.TileContext(nc) as tc, contextlib.ExitStack() as ctx:
        wpool = ctx.enter_context(tc.tile_pool(name="wpool", bufs=1))
        xin = ctx.enter_context(tc.tile_pool(name="xin", bufs=6))
        xinr = ctx.enter_context(tc.tile_pool(name="xinr", bufs=2))
        psum = ctx.enter_context(tc.tile_pool(name="psum", bufs=2, space="PSUM"))
        outp = ctx.enter_context(tc.tile_pool(name="outp", bufs=6))
        outr = ctx.enter_context(tc.tile_pool(name="outr", bufs=2))

        # PE warmup: ~20 junk matmuls with no data dependencies, issued
        # before any real work. They run during the DMA ramp (t~5-10us)
        # and lift the HAM clock gate to 2.4 GHz before the real stream
        # starts. Results land in a scratch psum slot and are discarded;
        # any garbage/NaN is overwritten later because every bank's first
        # real matmul runs with start=True.
        warm = wpool.tile([128, W], f16, tag="warm")
        nc.gpsimd.memset(warm[:], 0.0)
        psW = psum.tile([128, W], f32, tag="psA")
        for _ in range(20):
            nc.tensor.matmul(
                psW[:, :], warm[:, :128], warm[:, :],
                start=True, stop=True, skip_group_check=True,
            )

        t_wa = wpool.tile([128, 4 * 128], f16, tag="wa")
        nc.scalar.dma_start(out=t_wa[:], in_=d_wa)
        t_wb = wpool.tile([128, 4 * 128], f16, tag="wb")
        nc.scalar.dma_start(out=t_wb[:], in_=d_wb)
        t_wr = wpool.tile([RG * 6, 4 * RG * 4], f16, tag="wr")
        nc.scalar.dma_start(out=t_wr[:], in_=d_wr)

        def conv_mms(ps, wt, xt, xrows, hoff):
            """4 shifts x 4 oct-planes accumulating matmuls into the
            half-oct psum tile ps [128, 4*W] (2 banks; per-bank first mm
            gets start=True). Shift-major order so 4 consecutive matmuls
            share one stationary weight load. hoff selects planes
            [hoff, hoff+4)."""
            last = (SHIFT_ORDER[-1], 3)
            for i in SHIFT_ORDER:
                wl, wh, cl, ch = SHIFT_RANGES[i]
                lhsT = wt[:xrows, i * 128 : i * 128 + 128]
                for q in range(4):
                    nc.tensor.matmul(
                        ps[:128, q * W + wl : q * W + wh],
                        lhsT,
                        xt[:xrows, (hoff + q) * SEC + cl : (hoff + q) * SEC + ch],
                        start=(i == SHIFT_ORDER[0] and q % 2 == 0),
                        stop=((i, q) == last),
                        skip_group_check=True,
                    )

        def emit_half(o, ps, hoff, alt):
            """cast-copy psum half-oct tile [128, 4*W] f32 into fp16 out
            tile o at cols [hoff*W, (hoff+4)*W), split across engines."""
            hw = 2 * W
            base = hoff * W
            if alt:
                nc.scalar.copy(o[:, base : base + hw], ps[:, :hw])
                nc.vector.tensor_copy(o[:, base + hw : base + 4 * W], ps[:, hw:])
            else:
                nc.vector.tensor_copy(o[:, base : base + hw], ps[:, :hw])
                nc.scalar.copy(o[:, base + hw : base + 4 * W], ps[:, hw:])

        ri = 0
        for g in range(nquad):
            ta = xin.tile([128, Q * SEC], f16, tag="ta")
            if g == 0:
                # split the very first load by plane-halves so the first
                # matmul group (planes 0-3) starts as soon as possible
                nc.sync.dma_start(out=ta[:, 0 : 4 * SEC], in_=d_xs[g, 0:128, 0 : 4 * SEC])
                nc.sync.dma_start(out=ta[:, 4 * SEC :], in_=d_xs[g, 0:128, 4 * SEC :])
            else:
                nc.sync.dma_start(out=ta[:], in_=d_xs[g, 0:128, :])
            tb = xin.tile([128, Q * SEC], f16, tag="tb")
            nc.sync.dma_start(out=tb[:], in_=d_xs[g, 125:253, :])

            # the last oct's stores ride the sync ring (idle once loads done)
            ring = nc.sync if g == nquad - 1 else nc.scalar
            oa = outp.tile([128, Q * W], f16, tag="oa")
            for hoff in (0, 4):
                psA = psum.tile([128, 4 * W], f32, tag="psA")
                conv_mms(psA, t_wa, ta, 128, hoff)
                emit_half(oa, psA, hoff, alt=(g + hoff) % 2 == 0)
            ring.dma_start(out=d_out[g, 0:128, :], in_=oa[:])

            ob = outp.tile([128, Q * W], f16, tag="ob")
            for hoff in (0, 4):
                psB = psum.tile([128, 4 * W], f32, tag="psB")
                conv_mms(psB, t_wb, tb, 128, hoff)
                emit_half(ob, psB, hoff, alt=(g + hoff) % 2 == 1)
                if g == nquad - 1:
                    ring.dma_start(
                        out=d_out[g, 128:256, hoff * W : (hoff + 4) * W],
                        in_=ob[:, hoff * W : (hoff + 4) * W],
                    )
            if g != nquad - 1:
                ring.dma_start(out=d_out[g, 128:256, :], in_=ob[:])

            # stacked remainder: input rows come straight from DRAM, so
            # emit early (octs 2, 4, ...) to keep them off the kernel tail
            if ri < len(rem_groups) and g == min(2 * (ri + 1), nquad - 1):
                s, gsz = rem_groups[ri]
                ri += 1
                tr = xinr.tile([RG * 6, Q * SEC], f16, tag="tr")
                nc.sync.dma_start(
                    out=tr[: 6 * gsz, :], in_=d_xs[s : s + gsz, 250:256, :]
                )
                orr = outr.tile([RG * 4, Q * W], f16, tag="orr")
                for hoff in (0, 4):
                    psR = psum.tile([RG * 4, 4 * W], f32, tag="psA")
                    last = (SHIFT_ORDER[-1], 3)
                    for i in SHIFT_ORDER:
                        wl, wh, cl, ch = SHIFT_RANGES[i]
                        lhsT = t_wr[: 6 * gsz, i * RG * 4 : i * RG * 4 + 4 * gsz]
                        for q in range(4):
                            nc.tensor.matmul(
                                psR[: 4 * gsz, q * W + wl : q * W + wh],
                                lhsT,
                                tr[: 6 * gsz, (hoff + q) * SEC + cl : (hoff + q) * SEC + ch],
                                start=(i == SHIFT_ORDER[0] and q % 2 == 0),
                                stop=((i, q) == last),
                                skip_group_check=True,
                            )
                    if (g + hoff) % 2 == 0:
                        nc.scalar.copy(
                            orr[: 4 * gsz, hoff * W : (hoff + 4) * W], psR[: 4 * gsz, :]
                        )
                    else:
                        nc.vector.tensor_copy(
                            orr[: 4 * gsz, hoff * W : (hoff + 4) * W], psR[: 4 * gsz, :]
                        )
                nc.scalar.dma_start(
                    out=d_out[s : s + gsz, H : H + 4, :], in_=orr[: 4 * gsz, :]
                )

    nc.compile()
    return nc


_CACHE = {}


def _get_program(nquad: int = NQUAD):
    if nquad not in _CACHE:
        _CACHE[nquad] = _build_program(nquad)
    return _CACHE[nquad]


def _run(x: np.ndarray, wk: np.ndarray, trace: bool = False):
    """x: [P, 256, 256] f32 full stack of planes (P divisible by 8*Q),
    wk: flipped 4x4 kernel. Returns ([P, 256, 256] f32, exec_time_ns|None)."""
    P = x.shape[0]
    qper = P // (N_CORES * Q)
    xs = x.astype(np.float16)  # [P, 256, 256]
    # oct-pack: [P/Q, Q, H, SEC] -> [P/Q, H, Q, SEC] -> [P/Q, H, Q*SEC]
    xsq = (
        xs.reshape(P // Q, Q, H, SEC)
        .transpose(0, 2, 1, 3)
        .reshape(P // Q, H, Q * SEC)
    )

    wa, wb, wr = _make_weights(wk)
    nc = _get_program(qper)

    in_maps = [
        {
            "xs": np.ascontiguousarray(xsq[c * qper : (c + 1) * qper]),
            "wa": wa,
            "wb": wb,
            "wr": wr,
        }
        for c in range(N_CORES)
    ]
    res = run_bass_kernel_spmd(nc, in_maps, list(range(N_CORES)), trace=trace)
    outq = np.concatenate([r["out"] for r in res.results], axis=0)  # [P/Q, H+4, Q*W]
    outq = np.concatenate(
        [outq[:, 0:127], outq[:, 128:253], outq[:, 256:260]], axis=1
    )  # drop junk rows -> [P/Q, 256, Q*W]
    out = (
        outq.reshape(P // Q, H, Q, W)
        .transpose(0, 2, 1, 3)
        .reshape(P, H, W)
        .astype(np.float32)
    )
    return np.ascontiguousarray(out), res.exec_time_ns


def kernel(input: np.ndarray, kernel: np.ndarray) -> np.ndarray:
    x = np.asarray(input, dtype=np.float32)
    k = np.asarray(kernel, dtype=np.float32)
    n, c, h, w = x.shape
    wk = np.flip(k, (0, 1)).copy()  # correlation weights
    out, _ = _run(x.reshape(n * c, h, w), wk, trace=False)
    return out.reshape(n, c, h, w)


# revision 13
# speedup vs baseline: 1.0265x; 1.0044x over previous
"""Trainium2 Bass kernel for nn_Blur2: depthwise 4x4 blur (upfirdn2d-style,
pad=(2,1), unit stride) over input [8, 128, 256, 256] f32.

Strategy: pure data parallel over the 1024 independent (n, c) planes --
128 planes per NeuronCore. Within a plane the 2D 16-tap conv runs on the
tensor engine as banded matmuls: the H-direction conv is the contraction
(banded Toeplitz fp16 weights, image rows on partitions) and the
W-direction conv is 4 shifted slices of the moving operand accumulated
into the same PSUM tile (clipped column ranges encode the zero padding,
clipped weight bands encode the H padding).

Precision: tolerance is rel 2e-2 at output scale ~6.5, so the input is
quantized to a SINGLE fp16 (worst-case output err ~1.1e-2 abs, measured
~2e-3 rel) -- this halves both the matmul count and the input DMA
traffic vs an exact hi/lo split. The output is likewise stored as fp16
(adds <= 2^-11 rel) and upcast to f32 on host.

DMA: planes are packed in OCTS on host -- each DRAM row holds 8 planes'
fp16 data = 4 KB -- so every DMA moves >=4KB per partition, which is
the difference between ~100 GB/s and ~340 GB/s per core on TRN2. The
output uses a 260-row-per-oct DRAM layout (junk rows at 127, 253..255)
so both store DMAs are exactly 128 partitions: the HWDGE splits an
SBUF->DRAM DMA across its 16 SDMA engines only when the partition count
divides into 16 chunks. Loads go on the sync HWDGE ring, stores on the
scalar HWDGE ring; the last oct's stores ride the by-then-idle sync
ring to shorten the kernel tail.

Measured on 8 cores: HW exec ~134 us (PE ~87% busy, gapless at its
109 ns/matmul streaming floor for 1056 N~255 matmuls; ~11.5 us fixed
startup preamble + ~5 us tail), max rel err 5.2e-4 vs the f32 oracle
(tolerance 2e-2). Post-mortems of rejected faster-looking schemes: a
separable two-stage formulation (H-conv then W-conv on y^T, 2x fewer
streamed columns) measured 190-193 us because every matmul swaps its
stationary operand and each LDWEIGHTS is ~98 ns SERIAL with this
toolchain (walrus runs with --enable-ldw-opt=false, so the background
weight buffer is never used) plus HAM clock oscillation from PE
micro-idles; fp8-single fails the tolerance; DVE transposes cannot
cross 32-partition banks. Shift-major matmul order (one stationary
shared by 4 consecutive matmuls) is ~2 us faster than quad-major.
"""
import sys

for _p in ("/opt/trn_rl_repo", "/opt/pypackages"):
    if _p not in sys.path:
        sys.path.insert(0, _p)

import contextlib

import numpy as np


def _install_ntff_hook_shim():
    """The agent image's antenv lacks axon_hooks, which bass_utils needs
    for trace=True under axon. Provide it in sys.modules, backed by
    trn_agent_boot's ctypes NTFF shim."""
    import types

    if "antenv.axon_hooks" in sys.modules:
        return
    mod = types.ModuleType("antenv.axon_hooks")
    state = {"hook": None, "tried": False}

    def set_axon_ntff_profile_hook(hook):
        state["hook"] = hook

    def get_axon_ntff_profile_hook():
        if state["hook"] is None and not state["tried"]:
            state["tried"] = True
            try:
                from trn_agent_boot.trn_boot import _ntff_profile_via_ctypes

                state["hook"] = _ntff_profile_via_ctypes("/opt/axon/libaxon_pjrt.so")
            except Exception:
                state["hook"] = None
        return state["hook"]

    mod.set_axon_ntff_profile_hook = set_axon_ntff_profile_hook
    mod.get_axon_ntff_profile_hook = get_axon_ntff_profile_hook
    sys.modules["antenv.axon_hooks"] = mod
    try:
        import antenv

        antenv.axon_hooks = mod
    except ImportError:
        pass


_install_ntff_hook_shim()

import concourse.bacc as bacc
import concourse.tile as tile
from concourse import mybir
from concourse.bass_utils import run_bass_kernel_spmd

N_CORES = 8
H = W = 256
PLANES = 1024 // N_CORES  # 128 per core
Q = 8  # planes packed per SBUF/DRAM row (4 KB of fp16)
NQUAD = PLANES // Q  # 16 oct-groups per core
SEC = W  # one plane's section in a packed row: 256 fp16

# M-tile layout along H per plane:
#   tile A: out rows [0, 127)   from x rows [0, 128)
#   tile B: out rows [127, 252) from x rows [125, 253)
#   remainder: out rows [252, 256) from x rows [250, 256), stacked across
#   groups of RG=16 octs (96 partitions, 64 out rows per plane-slot)
MA, MB = 127, 125
RG = 16

# per W-shift i: out cols [wl, wh), reading x cols [cl, ch)  (tap = w-2+i)
SHIFT_RANGES = {
    0: (2, 256, 0, 254),
    1: (1, 256, 0, 255),
    2: (0, 256, 0, 256),
    3: (0, 255, 1, 256),
}
SHIFT_ORDER = [2, 0, 1, 3]  # full-range shift first so start=True covers all


def _make_weights(wk: np.ndarray):
    """wk: flipped 4x4 kernel. Packed fp16 weights, one 128-col matrix per
    W-shift (cols padded with zeros past MA/MB so NumWeights==128 enables
    the PE Fast-Weight-Load path): wa/wb [128, 4*128], wr [96, 4*64]
    (block-diag 16x(6->4))."""
    wa = np.zeros((128, 4, 128), np.float32)
    for k in range(128):
        for m in range(MA):
            d = k - m + 2
            if 0 <= d <= 3:
                wa[k, :, m] = wk[d, :]
    wb = np.zeros((128, 4, 128), np.float32)
    for k in range(128):
        for m in range(MB):
            d = k - m
            if 0 <= d <= 3:
                wb[k, :, m] = wk[d, :]
    wr = np.zeros((RG * 6, 4, RG * 4), np.float32)
    for b in range(RG):
        for r in range(6):
            for c in range(4):
                d = r - c
                if 0 <= d <= 3:
                    wr[6 * b + r, :, 4 * b + c] = wk[d, :]
    return (
        wa.reshape(128, 4 * 128).astype(np.float16),
        wb.reshape(128, 4 * 128).astype(np.float16),
        wr.reshape(RG * 6, 4 * RG * 4).astype(np.float16),
    )


def _build_program(nquad: int = NQUAD):
    nc = bacc.Bacc("TRN2", target_bir_lowering=False, debug=False)
    f16, f32 = mybir.dt.float16, mybir.dt.float32

    d_xs = nc.dram_tensor("xs", [nquad, H, Q * SEC], f16, kind="ExternalInput").ap()
    d_wa = nc.dram_tensor("wa", [128, 4 * 128], f16, kind="ExternalInput").ap()
    d_wb = nc.dram_tensor("wb", [128, 4 * 128], f16, kind="ExternalInput").ap()
    d_wr = nc.dram_tensor("wr", [RG * 6, 4 * RG * 4], f16, kind="ExternalInput").ap()
    d_out = nc.dram_tensor("out", [nquad, H + 4, Q * W], f16, kind="ExternalOutput").ap()

    rem_groups = [(s, min(RG, nquad - s)) for s in range(0, nquad, RG)]

    with tile.TileContext(nc) as tc, contextlib.ExitStack() as ctx:
        wpool = ctx.enter_context(tc.tile_pool(name="wpool", bufs=1))
        xin = ctx.enter_context(tc.tile_pool(name="xin", bufs=6))
        xinr = ctx.enter_context(tc.tile_pool(name="xinr", bufs=2))
        psum = ctx.enter_context(tc.tile_pool(name="psum", bufs=2, space="PSUM"))
        outp = ctx.enter_context(tc.tile_pool(name="outp", bufs=6))
        outr = ctx.enter_context(tc.tile_pool(name="outr", bufs=2))

        # PE warmup: ~20 junk matmuls with no data dependencies, issued
        # before any real work. They run during the DMA ramp (t~5-10us)
        # and lift the HAM clock gate to 2.4 GHz before the real stream
        # starts. Results land in a scratch psum slot and are discarded;
        # any garbage/NaN is overwritten later because every bank's first
        # real matmul runs with start=True.
        warm = wpool.tile([128, W], f16, tag="warm")
        nc.gpsimd.memset(warm[:], 0.0)
        psW = psum.tile([128, W], f32, tag="psA")
        for _ in range(10):
            nc.tensor.matmul(
                psW[:, :], warm[:, :128], warm[:, :],
                start=True, stop=True, skip_group_check=True,
            )

        t_wa = wpool.tile([128, 4 * 128], f16, tag="wa")
        nc.scalar.dma_start(out=t_wa[:], in_=d_wa)
        t_wb = wpool.tile([128, 4 * 128], f16, tag="wb")
        nc.scalar.dma_start(out=t_wb[:], in_=d_wb)
        t_wr = wpool.tile([RG * 6, 4 * RG * 4], f16, tag="wr")
        nc.scalar.dma_start(out=t_wr[:], in_=d_wr)

        def conv_mms(ps, wt, xt, xrows, hoff):
            """4 shifts x 4 oct-planes accumulating matmuls into the
            half-oct psum tile ps [128, 4*W] (2 banks). Shift-major order
            so consecutive matmuls share one stationary weight load; the
            two planes sharing a psum bank are fused into ONE matmul via
            a 2-level free-dim AP (planes are equi-strided in SBUF and
            PSUM), halving the matmul count. hoff selects planes
            [hoff, hoff+4)."""
            last = (SHIFT_ORDER[-1], 1)
            for i in SHIFT_ORDER:
                wl, wh, cl, ch = SHIFT_RANGES[i]
                lhsT = wt[:xrows, i * 128 : i * 128 + 128]
                for h in range(2):
                    rhs = xt[
                        :xrows, (hoff + 2 * h) * SEC : (hoff + 2 * h + 2) * SEC
                    ].rearrange("p (q w) -> p q w", q=2)[:, :, cl:ch]
                    out = ps[
                        :128, h * 2 * W : (h + 1) * 2 * W
                    ].rearrange("p (q w) -> p q w", q=2)[:, :, wl:wh]
                    nc.tensor.matmul(
                        out, lhsT, rhs,
                        start=(i == SHIFT_ORDER[0]),
                        stop=((i, h) == last),
                        skip_group_check=True,
                    )

        def emit_half(o, ps, hoff, alt):
            """cast-copy psum half-oct tile [128, 4*W] f32 into fp16 out
            tile o at cols [hoff*W, (hoff+4)*W), split across engines."""
            hw = 2 * W
            base = hoff * W
            if alt:
                nc.scalar.copy(o[:, base : base + hw], ps[:, :hw])
                nc.vector.tensor_copy(o[:, base + hw : base + 4 * W], ps[:, hw:])
            else:
                nc.vector.tensor_copy(o[:, base : base + hw], ps[:, :hw])
                nc.scalar.copy(o[:, base + hw : base + 4 * W], ps[:, hw:])

        ri = 0
        for g in range(nquad):
            ta = xin.tile([128, Q * SEC], f16, tag="ta")
            if g == 0:
                # split the very first load by plane-halves so the first
                # matmul group (planes 0-3) starts as soon as possible
                nc.sync.dma_start(out=ta[:, 0 : 4 * SEC], in_=d_xs[g, 0:128, 0 : 4 * SEC])
                nc.sync.dma_start(out=ta[:, 4 * SEC :], in_=d_xs[g, 0:128, 4 * SEC :])
            else:
                nc.sync.dma_start(out=ta[:], in_=d_xs[g, 0:128, :])
            tb = xin.tile([128, Q * SEC], f16, tag="tb")
            nc.sync.dma_start(out=tb[:], in_=d_xs[g, 125:253, :])

            # the last oct's stores ride the sync ring (idle once loads done)
            ring = nc.sync if g == nquad - 1 else nc.scalar
            oa = outp.tile([128, Q * W], f16, tag="oa")
            for hoff in (0, 4):
                psA = psum.tile([128, 4 * W], f32, tag="psA")
                conv_mms(psA, t_wa, ta, 128, hoff)
                emit_half(oa, psA, hoff, alt=(g + hoff) % 2 == 0)
            ring.dma_start(out=d_out[g, 0:128, :], in_=oa[:])

            ob = outp.tile([128, Q * W], f16, tag="ob")
            for hoff in (0, 4):
                psB = psum.tile([128, 4 * W], f32, tag="psB")
                conv_mms(psB, t_wb, tb, 128, hoff)
                emit_half(ob, psB, hoff, alt=(g + hoff) % 2 == 1)
                if g == nquad - 1:
                    ring.dma_start(
                        out=d_out[g, 128:256, hoff * W : (hoff + 4) * W],
                        in_=ob[:, hoff * W : (hoff + 4) * W],
                    )
            if g != nquad - 1:
                ring.dma_start(out=d_out[g, 128:256, :], in_=ob[:])

            # stacked remainder: input rows come straight from DRAM, so
            # emit early (octs 2, 4, ...) to keep them off the kernel tail
            if ri < len(rem_groups) and g == min(2 * (ri + 1), nquad - 1):
                s, gsz = rem_groups[ri]
                ri += 1
                tr = xinr.tile([RG * 6, Q * SEC], f16, tag="tr")
                nc.sync.dma_start(
                    out=tr[: 6 * gsz, :], in_=d_xs[s : s + gsz, 250:256, :]
                )
                orr = outr.tile([RG * 4, Q * W], f16, tag="orr")
                for hoff in (0, 4):
                    psR = psum.tile([RG * 4, 4 * W], f32, tag="psA")
                    last = (SHIFT_ORDER[-1], 3)
                    for i in SHIFT_ORDER:
                        wl, wh, cl, ch = SHIFT_RANGES[i]
                        lhsT = t_wr[: 6 * gsz, i * RG * 4 : i * RG * 4 + 4 * gsz]
                        for q in range(4):
                            nc.tensor.matmul(
                                psR[: 4 * gsz, q * W + wl : q * W + wh],
                                lhsT,
                                tr[: 6 * gsz, (hoff + q) * SEC + cl : (hoff + q) * SEC + ch],
                                start=(i == SHIFT_ORDER[0] and q % 2 == 0),
                                stop=((i, q) == last),
                                skip_group_check=True,
                            )
                    if (g + hoff) % 2 == 0:
                        nc.scalar.copy(
                            orr[: 4 * gsz, hoff * W : (hoff + 4) * W], psR[: 4 * gsz, :]
                        )
                    else:
                        nc.vector.tensor_copy(
                            orr[: 4 * gsz, hoff * W : (hoff + 4) * W], psR[: 4 * gsz, :]
                        )
                nc.scalar.dma_start(
                    out=d_out[s : s + gsz, H : H + 4, :], in_=orr[: 4 * gsz, :]
                )

    nc.compile()
    return nc


_CACHE = {}


def _get_program(nquad: int = NQUAD):
    if nquad not in _CACHE:
        _CACHE[nquad] = _build_program(nquad)
    return _CACHE[nquad]


def _run(x: np.ndarray, wk: np.ndarray, trace: bool = False):
    """x: [P, 256, 256] f32 full stack of planes (P divisible by 8*Q),
    wk: flipped 4x4 kernel. Returns ([P, 256, 256] f32, exec_time_ns|None)."""
    P = x.shape[0]
    qper = P // (N_CORES * Q)
    xs = x.astype(np.float16)  # [P, 256, 256]
    # oct-pack: [P/Q, Q, H, SEC] -> [P/Q, H, Q, SEC] -> [P/Q, H, Q*SEC]
    xsq = (
        xs.reshape(P // Q, Q, H, SEC)
        .transpose(0, 2, 1, 3)
        .reshape(P // Q, H, Q * SEC)
    )

    wa, wb, wr = _make_weights(wk)
    nc = _get_program(qper)

    in_maps = [
        {
            "xs": np.ascontiguousarray(xsq[c * qper : (c + 1) * qper]),
            "wa": wa,
            "wb": wb,
            "wr": wr,
        }
        for c in range(N_CORES)
    ]
    res = run_bass_kernel_spmd(nc, in_maps, list(range(N_CORES)), trace=trace)
    outq = np.concatenate([r["out"] for r in res.results], axis=0)  # [P/Q, H+4, Q*W]
    outq = np.concatenate(
        [outq[:, 0:127], outq[:, 128:253], outq[:, 256:260]], axis=1
    )  # drop junk rows -> [P/Q, 256, Q*W]
    out = (
        outq.reshape(P // Q, H, Q, W)
        .transpose(0, 2, 1, 3)
        .reshape(P, H, W)
        .astype(np.float32)
    )
    return np.ascontiguousarray(out), res.exec_time_ns


def kernel(input: np.ndarray, kernel: np.ndarray) -> np.ndarray:
    x = np.asarray(input, dtype=np.float32)
    k = np.asarray(kernel, dtype=np.float32)
    n, c, h, w = x.shape
    wk = np.flip(k, (0, 1)).copy()  # correlation weights
    out, _ = _run(x.reshape(n * c, h, w), wk, trace=False)
    return out.reshape(n, c, h, w)
